# revision 1
# baseline (speedup 1.0000x reference)
"""Distributed Trainium2 kernel for AGGEdgeGraph message passing.

Reference:
    t = edge_feats @ W + b                      # [E, H]
    out[i] = t[i] + sum_k t[neighbors[i, k]]    # [E, H]

Strategy (8 NeuronCores, edge-sharded, E/8 = 12500 -> padded 12800/core):
  - Phase 1: per-core tiled matmul (PE, bf16) -> t shard (bf16) -> DRAM.
  - Local gather: dma_gather rows of the core's own t shard for neighbor
    tokens that live locally (issued before the AllGather so it overlaps).
  - Phase 2: AllGather t shards -> full table [8*12800, 128] bf16 in DRAM.
  - Remote gather: dma_gather from the table. dma_gather indices are int16
    (< 32768), so the 102400-row table is addressed as 4 quarters with
    per-quarter index lists (per-call base offset).
  - Reduce: token order within a call is arbitrary; each gathered chunk of
    128 tokens is reduced into its 256-edge window via PE matmul with an
    on-device selector matrix S[tok, edge] = (dest_id[tok] == iota_edge),
    accumulating psum[h, edge]. Dummy padding tokens carry dest -1 -> zero
    column. Then PE-transpose, add self term t[i], DMA out (f32).
  - The SPMD graph is shared by all 8 cores: per-call sizes are the max
    over cores (data-derived), with per-core dummy padding.
"""

import sys

if "/opt/trn_rl_repo" not in sys.path:
    sys.path.insert(0, "/opt/trn_rl_repo")

import numpy as np
import ml_dtypes

import concourse.bass as bass
import concourse.bacc as bacc
import concourse.mybir as mybir
import concourse.tile as tile
from concourse.bass_utils import run_bass_kernel_spmd
from concourse.masks import make_identity

NCORES = 8
F = 256
H = 128
K = 16

E_FULL = 100000
EPC_RAW = E_FULL // NCORES   # 12500
EPC = 12800                  # padded (multiple of WIN and 128)
WIN = 512                    # edges per reduce window (= S width, psum free)
NRANGE = 5                   # table row-ranges (each < 32768 for int16 idx)
MAXCALL = 1024               # dma_gather index cap on this runtime

BF16 = mybir.dt.bfloat16
F32 = mybir.dt.float32
NP_BF16 = ml_dtypes.bfloat16


# ---------------------------------------------------------------------------
# Host-side planning: uniform call structure across cores + packed arrays.
# ---------------------------------------------------------------------------

def plan(neighbors, epc=EPC, ncores=NCORES, epc_raw=None):
    """Build the shared call plan and per-core packed index/dest arrays.

    Returns (calls, per_core) where
      calls: list of dicts(kind, win, size, off16, offc) — kind 0 = local
             shard, 1..4 = table quarter (kind-1).
      per_core: list of dicts with 'idx' [128, tot16] int16 and
             'dest' [128, totc] bf16 arrays.
    """
    if epc_raw is None:
        epc_raw = epc * E_FULL // (epc * ncores) if False else EPC_RAW
    e_full = neighbors.shape[0]
    epc_raw = e_full // ncores
    nwin = epc // WIN
    trows = ncores * epc
    qrows = -(-trows // NRANGE)

    # per-core token lists grouped by (window, kind)
    grouped = []  # [core][win][kind] -> (idx_vals int64, dest int64)
    for c in range(ncores):
        nb = np.asarray(neighbors[c * epc_raw:(c + 1) * epc_raw], np.int64)
        v = epc * (nb // epc_raw) + (nb % epc_raw)      # padded-global rows
        e = np.repeat(np.arange(epc_raw, dtype=np.int64), K)
        v = v.reshape(-1)
        w = e // WIN
        dest = e % WIN
        is_local = (v // epc) == c
        half = (v - c * epc) // (epc // 2)
        kind = np.where(is_local, half, 2 + v // qrows)
        base = np.where(
            is_local, c * epc + half * (epc // 2), (v // qrows) * qrows
        )
        idxv = v - base
        order = np.lexsort((e, kind, w))
        w_s, k_s, i_s, d_s = w[order], kind[order], idxv[order], dest[order]
        core_g = [[None] * (NRANGE + 2) for _ in range(nwin)]
        # boundaries
        wk = w_s * 16 + k_s
        cuts = np.flatnonzero(np.diff(wk)) + 1
        starts = np.concatenate([[0], cuts])
        ends = np.concatenate([cuts, [wk.size]])
        for s0, e0 in zip(starts, ends):
            core_g[int(w_s[s0])][int(k_s[s0])] = (i_s[s0:e0], d_s[s0:e0])
        grouped.append(core_g)

    # uniform sizes: per (win, kind) max count over cores, ceil to 128,
    # split into <= MAXCALL pieces
    calls = []
    off16 = 0
    offc = 0
    for wn in range(nwin):
        for kind in range(NRANGE + 2):
            mx = 0
            for c in range(ncores):
                g = grouped[c][wn][kind]
                if g is not None:
                    mx = max(mx, g[0].size)
            if mx == 0:
                continue
            size = -(-mx // 16) * 16
            pieces = []
            left = size
            while left > 0:
                p = min(left, MAXCALL)
                pieces.append(p)
                left -= p
            for p in pieces:
                calls.append({
                    "kind": kind, "win": wn, "size": p,
                    "off16": off16, "offc": offc, "ci": len(calls),
                })
                off16 += p // 16
                offc += -(-p // 128)

    tot16, totc = off16, offc

    ncalls = len(calls)
    per_core = []
    for c in range(ncores):
        idx_arr = np.full((128, tot16), -1, np.int16)
        cnt_arr = np.zeros((1, ncalls), np.int32)
        dest_arr = np.full((128, totc), -1, np.float32)
        # fill per (win, kind) across its (possibly split) calls
        cursor = {}
        for cl in calls:
            key = (cl["win"], cl["kind"])
            cursor.setdefault(key, []).append(cl)
        for (wn, kind), cls in cursor.items():
            g = grouped[c][wn][kind]
            if g is None:
                iv = np.zeros(0, np.int64)
                dv = np.zeros(0, np.int64)
            else:
                iv, dv = g
            pos = 0
            for ci_, cl in enumerate(cls):
                n = cl["size"]
                take = max(0, min(n, iv.size - pos))
                vi = np.full(n, -1, np.int64)      # pad idx = -1 (trimmed)
                vd = np.full(n, -1.0, np.float64)
                if take:
                    vi[:take] = iv[pos:pos + take]
                    vd[:take] = dv[pos:pos + take]
                else:
                    vi[0] = 0  # keep one valid idx; dest -1 discards it
                pos += take
                cnt_arr[0, cl["ci"]] = max(take, 1)
                # idx pack: position i -> [i % 16, off16 + i//16], x8 cores
                blk = vi.astype(np.int16).reshape(n // 16, 16).T
                idx_arr[:, cl["off16"]:cl["off16"] + n // 16] = np.tile(
                    blk, (8, 1)
                )
                # dest pack: chunk k partition p = token k*128+p
                nch = -(-n // 128)
                vdp = np.full(nch * 128, -1.0, np.float64)
                vdp[:n] = vd
                dm = vdp.reshape(nch, 128).T
                dest_arr[:, cl["offc"]:cl["offc"] + nch] = dm
        per_core.append({
            "idx": idx_arr,
            "cnt": cnt_arr,
            "dest": np.ascontiguousarray(dest_arr).astype(np.int16),
        })
    return calls, per_core, tot16, totc


# ---------------------------------------------------------------------------
# Graph
# ---------------------------------------------------------------------------

def build_graph(calls, tot16, totc, epc=EPC, ncores=NCORES):
    nwin = epc // WIN
    nt = epc // 128
    trows = ncores * epc
    qrows = -(-trows // NRANGE)
    add = mybir.AluOpType.add

    nc = bacc.Bacc(
        "TRN2", target_bir_lowering=False, debug=False, num_devices=ncores
    )

    xt_d = nc.dram_tensor("xt", [nt // 4, 128, 1024], BF16,
                          kind="ExternalInput")
    w_d = nc.dram_tensor("w", [128, 256], BF16, kind="ExternalInput")
    b_d = nc.dram_tensor("bb", [128, 128], F32, kind="ExternalInput")
    idx_d = nc.dram_tensor("idx", [128, tot16], mybir.dt.int16,
                           kind="ExternalInput")
    dest_d = nc.dram_tensor("dst", [128, totc], mybir.dt.int16,
                            kind="ExternalInput")
    cnt_d = nc.dram_tensor("cnt", [1, max(1, len(calls))], mybir.dt.int32,
                           kind="ExternalInput")
    out_d = nc.dram_tensor("out", [epc, H], F32, kind="ExternalOutput")

    ag_in = nc.dram_tensor("ag_in", [epc, H], BF16)
    table = nc.dram_tensor("table", [trows, H], BF16, addr_space="Shared")

    # group calls by window for the consume stage
    by_win = [[] for _ in range(nwin)]
    for cl in calls:
        by_win[cl["win"]].append(cl)
    local_lo = [cl for cl in calls if cl["kind"] == 0]
    local_hi = [cl for cl in calls if cl["kind"] == 1]
    remote_calls = [cl for cl in calls if cl["kind"] >= 2]

    with tile.TileContext(nc) as tc:
        with (
            tc.tile_pool(name="const", bufs=1) as constp,
            tc.tile_pool(name="xt", bufs=3) as xtp,
            tc.tile_pool(name="ps1", bufs=2, space="PSUM") as ps1p,
            tc.tile_pool(name="tt", bufs=3) as ttp,
            tc.tile_pool(name="gloc", bufs=1) as glocp,
            tc.tile_pool(name="grem", bufs=6) as gremp,
            tc.tile_pool(name="S", bufs=4) as sp,
            tc.tile_pool(name="psw", bufs=2, space="PSUM") as pswp,
            tc.tile_pool(name="ev", bufs=2) as evp,
            tc.tile_pool(name="pst", bufs=2, space="PSUM") as pstp,
            tc.tile_pool(name="tself", bufs=3) as tselfp,
            tc.tile_pool(name="ot", bufs=3) as otp,
        ):
            # constants
            w_t = constp.tile([128, 256], BF16)
            nc.sync.dma_start(out=w_t[:, :], in_=w_d[:, :])
            b_t = constp.tile([128, 128], F32)
            nc.sync.dma_start(out=b_t[:, :], in_=b_d[:, :])
            idx_t = constp.tile([128, tot16], mybir.dt.int16)
            nc.sync.dma_start(out=idx_t[:, :], in_=idx_d[:, :])
            dest_t = constp.tile([128, totc], mybir.dt.int16)
            nc.sync.dma_start(out=dest_t[:, :], in_=dest_d[:, :])
            cnt_t = constp.tile([1, max(1, len(calls))], mybir.dt.int32)
            nc.sync.dma_start(out=cnt_t[:, :], in_=cnt_d[:, :])
            creg = nc.gpsimd.alloc_register("gather_cnt")
            ident = constp.tile([128, 128], F32)
            make_identity(nc, ident[:, :])
            iota_t = constp.tile([128, WIN], mybir.dt.int16)
            nc.gpsimd.iota(iota_t[:, :], pattern=[[1, WIN]], base=0,
                           channel_multiplier=0)

            # ---- Phase 1: t = X @ W + b (four 128-edge tiles per DMA) ----
            for i4 in range(nt // 4):
                xt_t = xtp.tile([128, 1024], BF16)
                nc.sync.dma_start(out=xt_t[:, :], in_=xt_d[i4, :, :])
                t_t = ttp.tile([128, 512], BF16)
                for h in range(4):
                    ps = ps1p.tile([128, 128], F32)
                    nc.tensor.matmul(
                        out=ps[:, :], lhsT=xt_t[:, h * 256:h * 256 + 128],
                        rhs=w_t[:, 0:128], start=True, stop=False,
                    )
                    nc.tensor.matmul(
                        out=ps[:, :],
                        lhsT=xt_t[:, h * 256 + 128:h * 256 + 256],
                        rhs=w_t[:, 128:256], start=False, stop=True,
                    )
                    nc.vector.tensor_tensor(
                        out=t_t[:, h * 128:(h + 1) * 128], in0=ps[:, :],
                        in1=b_t[:, :], op=add,
                    )
                nc.sync.dma_start(
                    out=ag_in[i4 * 512:(i4 + 1) * 512, :]
                    .rearrange("(h p) c -> p h c", h=4),
                    in_=t_t[:, :].rearrange("p (h c) -> p h c", h=4),
                )

            gtiles = {}

            def emit_local(cls_, lo, hi):
                for cl in cls_:
                    n = cl["size"]
                    nch = -(-n // 128)
                    g = glocp.tile([128, nch, 128], BF16,
                                   tag=f"gl{cl['off16']}")
                    nc.vector.memset(g[:, :, :], 0.0)
                    nc.gpsimd.reg_load(
                        creg, cnt_t[0:1, cl["ci"]:cl["ci"] + 1]
                    )
                    nc.gpsimd.dma_gather(
                        out_ap=g[:, :, :],
                        in_ap=ag_in[lo:hi, :],
                        idxs_ap=idx_t[:, cl["off16"]:cl["off16"] + n // 16],
                        num_idxs=n,
                        num_idxs_reg=creg,
                        elem_size=H,
                    )
                    gtiles[id(cl)] = g

            # local half-0 gathers start as soon as the first half of the
            # t shard is written; the AllGather trigger (needs the full
            # shard) is sandwiched so ncfw runs during local-half-1 calls.
            emit_local(local_lo, 0, epc // 2)
            nc.gpsimd.collective_compute(
                "AllGather",
                mybir.AluOpType.bypass,
                replica_groups=[list(range(ncores))],
                ins=[ag_in.ap().opt()],
                outs=[table.ap().opt()],
            )
            emit_local(local_hi, epc // 2, epc)

            # ---- Remote gathers ----
            for cl in remote_calls:
                n = cl["size"]
                q = cl["kind"] - 2
                nch = -(-n // 128)
                g = gremp.tile([128, MAXCALL // 128, 128], BF16)
                nc.vector.memset(g[:, 0:nch, :], 0.0)
                nc.gpsimd.reg_load(creg, cnt_t[0:1, cl["ci"]:cl["ci"] + 1])
                nc.gpsimd.dma_gather(
                    out_ap=g[:, 0:nch, :],
                    in_ap=table[q * qrows:min((q + 1) * qrows, trows), :],
                    idxs_ap=idx_t[:, cl["off16"]:cl["off16"] + n // 16],
                    num_idxs=n,
                    num_idxs_reg=creg,
                    elem_size=H,
                )
                gtiles[id(cl)] = g

            # ---- Reduce per window ----
            for wn in range(nwin):
                wcalls = by_win[wn]
                if not wcalls:
                    continue  # all-padding window: rows are discarded
                nch_total = sum(-(-cl["size"] // 128) for cl in wcalls)
                psw = pswp.tile([128, WIN], F32)
                ci = 0
                for cl in wcalls:
                    g = gtiles[id(cl)]
                    for k in range(-(-cl["size"] // 128)):
                        col = cl["offc"] + k
                        S = sp.tile([128, WIN], BF16)
                        nc.vector.tensor_tensor(
                            out=S[:, :],
                            in0=dest_t[:, col:col + 1]
                            .to_broadcast([128, WIN]),
                            in1=iota_t[:, :],
                            op=mybir.AluOpType.is_equal,
                        )
                        nc.tensor.matmul(
                            out=psw[:, :],
                            lhsT=g[:, k, :],
                            rhs=S[:, :],
                            start=(ci == 0),
                            stop=(ci == nch_total - 1),
                        )
                        ci += 1
                ev = evp.tile([128, WIN], F32)
                nc.vector.tensor_copy(out=ev[:, :], in_=psw[:, :])
                for j in range(WIN // 128):
                    pst = pstp.tile([128, 128], F32)
                    nc.tensor.transpose(
                        out=pst[:, :], in_=ev[:, j * 128:(j + 1) * 128],
                        identity=ident[:, :],
                    )
                    row0 = wn * WIN + j * 128
                    tself = tselfp.tile([128, 128], BF16)
                    nc.sync.dma_start(
                        out=tself[:, :], in_=ag_in[row0:row0 + 128, :]
                    )
                    ot = otp.tile([128, 128], F32)
                    nc.vector.tensor_tensor(
                        out=ot[:, :], in0=pst[:, :], in1=tself[:, :], op=add
                    )
                    nc.sync.dma_start(
                        out=out_d[row0:row0 + 128, :], in_=ot[:, :]
                    )

    nc.compile()
    return nc


# ---------------------------------------------------------------------------
# Host prep (matmul inputs) + entry point
# ---------------------------------------------------------------------------

def prep_core_mm(edge_feats, W, b, c, epc=EPC, ncores=NCORES):
    e_full = edge_feats.shape[0]
    epc_raw = e_full // ncores
    nt = epc // 128
    lo, hi = c * epc_raw, (c + 1) * epc_raw

    x = np.zeros((epc, F), np.float32)
    x[: epc_raw] = edge_feats[lo:hi]
    xt = np.ascontiguousarray(x.T)                       # [256, epc]
    xt4 = xt.reshape(2, 128, nt, 128)                    # [cc, k, i, e]
    per_tile = xt4.transpose(2, 1, 0, 3).reshape(nt, 128, 256)
    xt_arr = np.ascontiguousarray(
        per_tile.reshape(nt // 4, 4, 128, 256)
        .transpose(0, 2, 1, 3).reshape(nt // 4, 128, 1024)
    ).astype(NP_BF16)

    w_arr = np.ascontiguousarray(
        W.reshape(2, 128, H).transpose(1, 0, 2).reshape(128, 256)
    ).astype(NP_BF16)
    bb = np.ascontiguousarray(np.broadcast_to(b, (128, H))).astype(np.float32)
    return {"xt": xt_arr, "w": w_arr, "bb": bb}


_CACHE = {}


def _get(neighbors, epc=EPC, ncores=NCORES):
    key = (epc, ncores, hash(neighbors.tobytes()))
    if key not in _CACHE:
        calls, per_core, tot16, totc = plan(neighbors, epc, ncores)
        nc = build_graph(calls, tot16, totc, epc, ncores)
        _CACHE.clear()
        _CACHE[key] = (nc, per_core)
    return _CACHE[key]


def make_in_maps(edge_feats, neighbors, W, b, epc=EPC, ncores=NCORES):
    nc, per_core = _get(neighbors, epc, ncores)
    in_maps = []
    for c in range(ncores):
        m = prep_core_mm(edge_feats, W, b, c, epc, ncores)
        m["idx"] = per_core[c]["idx"]
        m["dst"] = per_core[c]["dest"]
        m["cnt"] = per_core[c]["cnt"]
        in_maps.append(m)
    return nc, in_maps


def kernel(edge_feats, neighbors, W, b):
    edge_feats = np.asarray(edge_feats, np.float32)
    neighbors = np.asarray(neighbors, np.int32)
    W = np.asarray(W, np.float32)
    b = np.asarray(b, np.float32)
    e_full = edge_feats.shape[0]
    epc_raw = e_full // NCORES

    nc, in_maps = make_in_maps(edge_feats, neighbors, W, b)
    res = run_bass_kernel_spmd(nc, in_maps, core_ids=list(range(NCORES)))
    shards = [
        np.asarray(res.results[c]["out"][:epc_raw], np.float32)
        for c in range(NCORES)
    ]
    return np.concatenate(shards, axis=0)

